# revision 1
# baseline (speedup 1.0000x reference)
"""MultiHeadCrossAttentionFusion kernel for TRN2 (8 NeuronCores, data-parallel over batch).

v3 layout strategy per core (batch shard BS=1024):
  Phase A: x -> xT via PE transposes packed 8-per-PSUM-bank with one DVE drain
           per bank; QKV matmuls (bf16) -> separate q/k/v DRAM tensors.
  Phase B: per 128-row tile / branch: partition-expansion DMAs load q/k/v packed
           ((sample, head) on partitions, feature free) straight from DRAM.
           Staged attention: paired PE transposes (2 sample-groups per 128x128)
           fill one PSUM bank, one DVE drain; 16 score matmuls; ACT exp straight
           from PSUM; block-diag 0/1 mask-mult on DVE; fused (v|1) matmul for
           numerator+denominator; normalize on DVE into a paired layout;
           paired transposes back; one drain; gpsimd scatter into persistent caT.
  Phase C: projection matmuls from caT with resident folded weights, pipelined
           per b-tile behind phase B; scale+residual fused in one DVE op.
"""
import sys
sys.path.insert(0, "/opt/trn_rl_repo")
import numpy as np
import ml_dtypes
from contextlib import ExitStack

import concourse.bass as bass
from concourse import bacc as _bacc
import concourse.bacc as _bacc_mod

_orig_get_tables = _bacc_mod.get_activation_tables


def _pref_tables(arch):
    # Keep set order/ids intact (walrus maps ids to act_info.json order);
    # hide Exp/Ln from all sets except the combined one so the first-fit
    # chooser lands on a single table set for the whole kernel.
    t = _orig_get_tables(arch)
    pref = "natural_log_exp_and_others"
    if pref in t:
        _AF = mybir.ActivationFunctionType
        for name, funcs in t.items():
            if name != pref:
                funcs.discard(_AF.Exp)
                funcs.discard(_AF.Ln)
    return t


_bacc_mod.get_activation_tables = _pref_tables
import concourse.mybir as mybir
from concourse.tile import TileContext
from concourse.bass_utils import run_bass_kernel_spmd

VEXT_FUSED = False      # strided-dst v DMA + ones col + single cu matmul
WG_ONESHOT = False      # single 4.2MB Wg DMA vs 4 chunks
SP_PAIRED = False       # paired sp bank + stride-0 mask broadcast

B, CD, HID, H, D = 8192, 2048, 1024, 16, 64
NCORES = 8
BS = B // NCORES          # 1024 rows per core
NB = BS // 128            # 8 b-tiles
KT = CD // 128            # 16 k-tiles for qkv matmul
NCH_Q = (3 * HID) // 512  # 6 n-chunks of qkv
CT = HID // 128           # 8 c-tiles for proj
NCH_P = CD // 512         # 4 n-chunks of proj
EPS = 1e-5
F32 = mybir.dt.float32
BF16 = mybir.dt.bfloat16
AL = mybir.AluOpType
AF = mybir.ActivationFunctionType


def build_nc(with_bias=True, linearize=False):
    nc = _bacc.Bacc()
    dp = nc.declare_dram_parameter
    x_c = dp("x_c", [BS, CD], F32, isOutput=False)
    x_m = dp("x_m", [BS, CD], F32, isOutput=False)
    Wq_c = dp("Wq_c", [CD, 3 * HID], BF16, isOutput=False)
    Wq_m = dp("Wq_m", [CD, 3 * HID], BF16, isOutput=False)
    bq_c = dp("bq_c", [1, 3 * HID], F32, isOutput=False)
    bq_m = dp("bq_m", [1, 3 * HID], F32, isOutput=False)
    Wg_c = dp("Wg_c", [HID, CD], BF16, isOutput=False)   # g1-folded, permuted proj W
    Wg_m = dp("Wg_m", [HID, CD], BF16, isOutput=False)
    v_c = dp("v_c", [1, CD], F32, isOutput=False)        # be1@Wp + b_proj
    v_m = dp("v_m", [1, CD], F32, isOutput=False)
    un_c = dp("un_c", [1, CD], BF16, isOutput=False)     # -(g-folded W).sum(0)
    un_m = dp("un_m", [1, CD], BF16, isOutput=False)
    mask01 = dp("mask01", [128, 128], BF16, isOutput=False)  # block-diag 1 / 0
    identb = dp("identb", [128, 128], BF16, isOutput=False)
    ones_bf = dp("ones_bf", [128, 1], BF16, isOutput=False)
    onesr_bf = dp("onesr_bf", [1, 128], BF16, isOutput=False)
    onesr_f = dp("onesr_f", [1, 128], F32, isOutput=False)
    out_c = dp("out_c", [BS, CD], F32, isOutput=True)
    out_m = dp("out_m", [BS, CD], F32, isOutput=True)

    with TileContext(nc, linearize=linearize) as tc, ExitStack() as ctx:
        consts = ctx.enter_context(tc.tile_pool(name="consts", bufs=1))
        dram = ctx.enter_context(tc.tile_pool(name="dram", bufs=1, space="DRAM"))
        psTT = ctx.enter_context(tc.tile_pool(name="psTT", bufs=2, space="PSUM"))
        psQ = ctx.enter_context(tc.tile_pool(name="psQ", bufs=2, space="PSUM"))
        psS = ctx.enter_context(tc.tile_pool(name="psS", bufs=2, space="PSUM"))
        psCA = ctx.enter_context(tc.tile_pool(name="psCA", bufs=2, space="PSUM"))

        # ---- constants
        sb_mask = consts.tile([128, 128], BF16)
        nc.sync.dma_start(sb_mask, mask01[:, :])
        sb_id = consts.tile([128, 128], BF16)
        nc.sync.dma_start(sb_id, identb[:, :])
        sb_ones = consts.tile([128, 1], BF16)
        nc.sync.dma_start(sb_ones, ones_bf[:, :])
        sb_or_bf = consts.tile([1, 128], BF16)
        nc.sync.dma_start(sb_or_bf, onesr_bf[:, :])
        sb_or_f = consts.tile([1, 128], F32)
        nc.sync.dma_start(sb_or_f, onesr_f[:, :])

        # ---- resident proj weights + LN-mu rows (+ bias rows), loaded early
        wgp = ctx.enter_context(tc.tile_pool(name="wgp", bufs=1))
        wgall, ung = {}, {}
        for t, Wgt, un in (("c", Wg_c, un_c), ("m", Wg_m, un_m)):
            w = wgp.tile([128, CT, CD], BF16, name=f"wg_{t}", tag=f"wg_{t}")
            if WG_ONESHOT:
                nc.sync.dma_start(
                    w, Wgt[:, :].rearrange("(ct p) n -> p ct n", p=128))
            else:
                for ch in range(NCH_P):
                    nc.sync.dma_start(
                        w[:, :, ch * 512:(ch + 1) * 512],
                        Wgt[:, ch * 512:(ch + 1) * 512].rearrange(
                            "(ct p) n -> p ct n", p=128))
            wgall[t] = w
            u = wgp.tile([1, CD], BF16, name=f"ung_{t}", tag=f"ung_{t}")
            nc.sync.dma_start(u, un[:, :])
            ung[t] = u
        sb_v = {}
        for t, vv in (("c", v_c), ("m", v_m)) if with_bias else ():
            row = wgp.tile([1, CD], F32, name=f"vr_{t}", tag=f"vr_{t}")
            nc.sync.dma_start(row, vv[:, :])
            sb_v[t] = wgp.tile([128, CD], F32, name=f"vb_{t}", tag=f"vb_{t}")
            for ch in range(NCH_P):
                vps = psQ.tile([128, 512], F32, tag="px", name="vps")
                nc.tensor.matmul(
                    vps, lhsT=sb_or_f,
                    rhs=row[0:1, ch * 512:(ch + 1) * 512],
                    start=True, stop=True)
                nc.scalar.copy(
                    out=sb_v[t][:, ch * 512:(ch + 1) * 512], in_=vps)

        # qkv natural-layout intermediates in DRAM (separate q/k/v tensors
        # so packed reads keep row stride == section width for AP merging)
        qkvd = {
            t: [dram.tile([BS, HID], BF16, name=f"{s}d_{t}", tag=f"{s}d_{t}")
                for s in ("q", "k", "v")]
            for t in ("c", "m")
        }

        # ---- Phase A: xT build + QKV matmuls
        pA_cm = tc.tile_pool(name="pA", bufs=1)
        pA = pA_cm.__enter__()
        tmpA_cm = tc.tile_pool(name="tmpA", bufs=2)
        tmpA = tmpA_cm.__enter__()
        wst_cm = tc.tile_pool(name="wstp", bufs=2)
        wstp = wst_cm.__enter__()

        xT = {
            "c": pA.tile([128, KT, BS], BF16, name="xT_c", tag="xT_c"),
            "m": pA.tile([128, KT, BS], BF16, name="xT_m", tag="xT_m"),
        }
        sb_bq = {}
        for t, bq in (("c", bq_c), ("m", bq_m)) if with_bias else ():
            row = pA.tile([1, 3 * HID], F32, name=f"bqr_{t}", tag=f"bqr_{t}")
            nc.sync.dma_start(row, bq[:, :])
            rowb = pA.tile([1, 3 * HID], BF16, name=f"bqrb_{t}",
                           tag=f"bqrb_{t}")
            nc.vector.tensor_copy(out=rowb, in_=row)
            sb_bq[t] = pA.tile([128, 3 * HID], BF16, name=f"bqb_{t}",
                               tag=f"bqb_{t}")
            for ch in range(NCH_Q):
                bps = psQ.tile([128, 512], F32, tag="px", name="bps")
                nc.tensor.matmul(
                    bps, lhsT=sb_or_bf,
                    rhs=rowb[0:1, ch * 512:(ch + 1) * 512],
                    start=True, stop=True)
                nc.scalar.copy(
                    out=sb_bq[t][:, ch * 512:(ch + 1) * 512], in_=bps)
        for t, xin in (("c", x_c), ("m", x_m)):
            for bt in range(NB):
                xn = tmpA.tile([128, CD], F32, tag="xn")
                nc.sync.dma_start(xn, xin[bt * 128:(bt + 1) * 128, :])
                xb = tmpA.tile([128, CD], BF16, tag="xb")
                nc.scalar.copy(out=xb, in_=xn)
                for half in range(2):
                    pb = psTT.tile([128, 8, 128], BF16, tag="ptb")
                    for i in range(8):
                        kt = half * 8 + i
                        nc.tensor.transpose(
                            pb[:, i, :], xb[:, kt * 128:(kt + 1) * 128], sb_id)
                    nc.vector.tensor_copy(
                        out=xT[t][:, half * 8:(half + 1) * 8,
                                  bt * 128:(bt + 1) * 128],
                        in_=pb)

        for t, Wt in (("c", Wq_c), ("m", Wq_m)):
            for nch in range(NCH_Q):
                wst = wstp.tile([128, KT, 512], BF16, tag="wst")
                nc.sync.dma_start(
                    wst,
                    Wt[:, nch * 512:(nch + 1) * 512].rearrange(
                        "(kt p) n -> p kt n", p=128))
                for bt in range(NB):
                    px = psQ.tile([128, 512], F32, tag="px")
                    for kt in range(KT):
                        nc.tensor.matmul(
                            px, lhsT=xT[t][:, kt, bt * 128:(bt + 1) * 128],
                            rhs=wst[:, kt, :],
                            start=(kt == 0), stop=(kt == KT - 1))
                    qb = tmpA.tile([128, 512], BF16, tag="qb")
                    if with_bias:
                        nc.vector.tensor_tensor(
                            out=qb, in0=px,
                            in1=sb_bq[t][:, nch * 512:(nch + 1) * 512],
                            op=AL.add)
                    else:
                        nc.vector.tensor_copy(out=qb, in_=px)
                    sect, scol = divmod(nch * 512, HID)
                    nc.sync.dma_start(
                        qkvd[t][sect][bt * 128:(bt + 1) * 128,
                                      scol:scol + 512], qb)

        wst_cm.__exit__(None, None, None)
        tmpA_cm.__exit__(None, None, None)
        pA_cm.__exit__(None, None, None)

        # ---- Phase B + C pools
        keep = ctx.enter_context(tc.tile_pool(name="keep", bufs=1))
        bpool = ctx.enter_context(tc.tile_pool(name="bpool", bufs=3))
        spool = ctx.enter_context(tc.tile_pool(name="spool", bufs=4))
        stp = ctx.enter_context(tc.tile_pool(name="stp", bufs=4))
        tmpC = ctx.enter_context(tc.tile_pool(name="tmpC", bufs=4))

        r_all = keep.tile([128, 2 * NB], F32, tag="r_all")
        mu_all = keep.tile([1, 2 * NB * 128], BF16, tag="mu_all")
        caT_all = keep.tile([128, 2 * NB * (H // 2), 128], BF16,
                            tag="caT_all")

        def phase_b_core(bt, bri, qs, ks):
            idx = bt * 2 + bri
            rows = slice(bt * 128, (bt + 1) * 128)
            qpack = bpool.tile([128, 16, D], BF16, tag="qpack")
            nc.sync.dma_start(
                qpack[:, :, :],
                qkvd[qs][0][rows, :].rearrange(
                    "(j b) (h d) -> b h j d", b=8, h=H))
            kpack = bpool.tile([128, 16, D], BF16, tag="kpack")
            nc.sync.dma_start(
                kpack[:, :, :],
                qkvd[ks][1][rows, :].rearrange(
                    "(j b) (h d) -> b h j d", b=8, h=H))
            if VEXT_FUSED:
                vext = bpool.tile([128, 16, D + 1], BF16, tag="vext")
                nc.sync.dma_start(
                    vext[:, :, 0:D],
                    qkvd[ks][2][rows, :].rearrange(
                        "(j b) (h d) -> b h j d", b=8, h=H))
                nc.gpsimd.memset(vext[:, :, D:D + 1], 1.0)
            else:
                vext = bpool.tile([128, 16, D], BF16, tag="vext")
                nc.sync.dma_start(
                    vext[:, :, :],
                    qkvd[ks][2][rows, :].rearrange(
                        "(j b) (h d) -> b h j d", b=8, h=H))

            # T stage: paired transposes, one PSUM bank + one drain per input
            qpkA = bpool.tile([128, 8, 128], BF16, tag="qpkA")
            kpkA = bpool.tile([128, 8, 128], BF16, tag="kpkA")
            for src, dstA in ((qpack, qpkA), (kpack, kpkA)):
                pb = psTT.tile([128, 8, 128], BF16, tag="ptb")
                for i in range(8):
                    nc.tensor.transpose(
                        pb[:, i, :],
                        src[:, 2 * i:2 * i + 2, :].rearrange(
                            "p j d -> p (j d)"),
                        sb_id)
                nc.vector.tensor_copy(out=dstA, in_=pb)

            caTi = caT_all[:, idx * (H // 2):(idx + 1) * (H // 2), :]
            eTa = bpool.tile([128, 16, 128], BF16, tag="eTa")
            cajA = bpool.tile([128, 8, 128], BF16, tag="cajA")
            drain_cwork(2)
            # S stage: score matmuls; exp straight from PSUM, mask after
            if SP_PAIRED:
                for i in range(8):
                    sp2 = psS.tile([128, 2, 128], F32, tag="sp")
                    for jp in range(2):
                        nc.tensor.matmul(
                            sp2[:, jp, :],
                            lhsT=kpkA[jp * D:(jp + 1) * D, i, :],
                            rhs=qpkA[jp * D:(jp + 1) * D, i, :],
                            start=True, stop=True)
                    eTr2 = spool.tile([128, 2, 128], BF16, tag="eTr")
                    nc.scalar.activation(eTr2, sp2, AF.Exp, scale=0.125)
                    mask_bc = bass.AP(
                        tensor=sb_mask.tensor, offset=sb_mask.offset,
                        ap=[list(sb_mask.ap[0]), [0, 2]] + list(sb_mask.ap)[1:])
                    nc.vector.tensor_tensor(
                        out=eTa[:, 2 * i:2 * i + 2, :], in0=eTr2, in1=mask_bc,
                        op=AL.mult)
            else:
                for j in range(16):
                    if j == 8:
                        drain_cwork(1)
                    jp, i = j % 2, j // 2
                    sp = psS.tile([128, 2, 128], F32, tag="sp")
                    nc.tensor.matmul(
                        sp[:, 0, :],
                        lhsT=kpkA[jp * D:(jp + 1) * D, i, :],
                        rhs=qpkA[jp * D:(jp + 1) * D, i, :],
                        start=True, stop=True)
                    eTr = spool.tile([128, 128], BF16, tag="eTr")
                    nc.scalar.activation(eTr, sp[:, 0, :], AF.Exp, scale=0.125)
                    nc.gpsimd.tensor_tensor(
                        out=eTa[:, j, :], in0=eTr, in1=sb_mask, op=AL.mult)
            drain_cwork(1)
            # U stage: fused (v|1) matmuls; N stage: normalize
            for j in range(16):
                if j == 8:
                    drain_cwork(1)
                jp, i = j % 2, j // 2
                cu = psCA.tile([128, D + 1], F32, tag="cu")
                if VEXT_FUSED:
                    nc.tensor.matmul(cu, lhsT=eTa[:, j, :], rhs=vext[:, j, :],
                                     start=True, stop=True)
                else:
                    nc.tensor.matmul(cu[:, 0:D], lhsT=eTa[:, j, :],
                                     rhs=vext[:, j, :], start=True, stop=True)
                    nc.tensor.matmul(cu[:, D:D + 1], lhsT=eTa[:, j, :],
                                     rhs=sb_ones, start=True, stop=True)
                rcz = stp.tile([128, 1], F32, tag="rcz")
                nc.vector.reciprocal(rcz, cu[:, D:D + 1])
                nc.vector.tensor_scalar(
                    out=cajA[:, i, jp * D:(jp + 1) * D],
                    in0=cu[:, 0:D], scalar1=rcz,
                    scalar2=None, op0=AL.mult)
            drain_cwork(1)
            # CT stage: paired transposes back, one bank + one drain
            ctb = psTT.tile([128, 8, 128], BF16, tag="ptb")
            for i in range(8):
                nc.tensor.transpose(ctb[:, i, :], cajA[:, i, :], sb_id)
            ctA = bpool.tile([128, 8, 128], BF16, tag="ctA")
            nc.vector.tensor_copy(out=ctA, in_=ctb)
            # SC stage: 4 bulk scatters into caT layout (2 on Pool, 2 on ACT)
            for jp in range(2):
                for par in range(2):
                    srcv = ctA[jp * D:(jp + 1) * D, :, :].rearrange(
                        "d i (b hp two) -> d i b hp two", b=8, two=2)[
                        :, :, :, :, par]
                    dstv = caTi[par * D:(par + 1) * D, :, :].rearrange(
                        "d hp (i jp2 b) -> d i b hp jp2", i=8, jp2=2)[
                        :, :, :, :, jp]
                    if par == 0:
                        nc.gpsimd.tensor_copy(out=dstv, in_=srcv)
                    else:
                        nc.scalar.copy(out=dstv, in_=srcv)
        def phase_b_stats(bt, bri):
            idx = bt * 2 + bri
            caTi = caT_all[:, idx * (H // 2):(idx + 1) * (H // 2), :]
            sq = spool.tile([128, H // 2, 128], BF16, tag="sqq")
            nc.vector.tensor_tensor(out=sq, in0=caTi, in1=caTi, op=AL.mult)
            mrow = psCA.tile([1, 128], F32, tag="cu")
            srow = psCA.tile([1, 128], F32, tag="cu")
            for hp in range(H // 2):
                nc.tensor.matmul(mrow, lhsT=sb_ones, rhs=caTi[:, hp, :],
                                 start=(hp == 0), stop=(hp == 7))
                nc.tensor.matmul(srow, lhsT=sb_ones, rhs=sq[:, hp, :],
                                 start=(hp == 0), stop=(hp == 7))
            murow = stp.tile([1, 128], F32, tag="murow")
            nc.vector.tensor_scalar(
                out=murow, in0=mrow, scalar1=1.0 / HID, scalar2=None,
                op0=AL.mult)
            mu2 = stp.tile([1, 128], F32, tag="mu2")
            nc.vector.tensor_tensor(out=mu2, in0=murow, in1=murow,
                                    op=AL.mult)
            vvr = stp.tile([1, 128], F32, tag="vvr")
            nc.vector.tensor_scalar(
                out=vvr, in0=srow, scalar1=1.0 / HID, scalar2=EPS,
                op0=AL.mult, op1=AL.add)
            vv2 = stp.tile([1, 128], F32, tag="vv2")
            nc.vector.tensor_tensor(out=vv2, in0=vvr, in1=mu2,
                                    op=AL.subtract)
            lnv = stp.tile([1, 128], F32, tag="lnv")
            nc.scalar.activation(lnv, vv2, AF.Ln)
            rrow = stp.tile([1, 128], F32, tag="rrow")
            nc.scalar.activation(rrow, lnv, AF.Exp, scale=-0.5)
            rrow_bf = stp.tile([1, 128], BF16, tag="rrow_bf")
            nc.vector.tensor_copy(out=rrow_bf, in_=rrow)
            rc_ps = psTT.tile([128, 1], BF16, tag="ptb")
            nc.tensor.transpose(rc_ps, rrow_bf, sb_id[0:1, 0:1])
            nc.scalar.copy(out=r_all[:, idx:idx + 1], in_=rc_ps)
            nc.vector.tensor_copy(
                out=mu_all[:, idx * 128:(idx + 1) * 128], in_=murow)

        cwork = []

        def phase_c_chunk(bt, bri, t, nch):
            idx = bt * 2 + bri
            xin = x_c if t == "c" else x_m
            outt = out_c if t == "c" else out_m
            if True:
                xres = tmpC.tile([128, 512], F32, tag="xres")
                nc.sync.dma_start(
                    xres, xin[bt * 128:(bt + 1) * 128,
                              nch * 512:(nch + 1) * 512])
                px = psQ.tile([128, 512], F32, tag="px")
                for ct in range(CT):
                    nc.tensor.matmul(
                        px, lhsT=caT_all[:, idx * CT + ct, :],
                        rhs=wgall[t][:, ct, nch * 512:(nch + 1) * 512],
                        start=(ct == 0), stop=False)
                nc.tensor.matmul(
                    px, lhsT=mu_all[:, idx * 128:(idx + 1) * 128],
                    rhs=ung[t][:, nch * 512:(nch + 1) * 512],
                    start=False, stop=True)
                t2 = tmpC.tile([128, 512], F32, tag="t2")
                nc.vector.scalar_tensor_tensor(
                    out=t2, in0=px, scalar=r_all[:, idx:idx + 1],
                    in1=xres, op0=AL.mult, op1=AL.add)
                if with_bias:
                    ot = tmpC.tile([128, 512], F32, tag="ot")
                    nc.vector.tensor_tensor(
                        out=ot, in0=t2,
                        in1=sb_v[t][:, nch * 512:(nch + 1) * 512],
                        op=AL.add)
                else:
                    ot = t2
                nc.sync.dma_start(
                    outt[bt * 128:(bt + 1) * 128,
                         nch * 512:(nch + 1) * 512], ot)

        def phase_c(bt, bri, t):
            for nch in range(NCH_P):
                cwork.append((bt, bri, t, nch))

        def drain_cwork(k):
            for _ in range(min(k, len(cwork))):
                phase_c_chunk(*cwork.pop(0))

        pending = []
        for bt in range(NB):
            for bri, (qs, ks) in ((0, ("c", "m")), (1, ("m", "c"))):
                phase_b_core(bt, bri, qs, ks)
                pending.append((bt, bri))
                if len(pending) > 1:
                    p = pending.pop(0)
                    phase_b_stats(*p)
                    if p[1] == 1:
                        phase_c(p[0], 0, "c")
                        phase_c(p[0], 1, "m")
        for p in pending:
            phase_b_stats(*p)
            if p[1] == 1:
                phase_c(p[0], 0, "c")
                phase_c(p[0], 1, "m")
        drain_cwork(len(cwork))
    return nc


_NC = {}


def _get_nc(with_bias):
    if with_bias not in _NC:
        nc = build_nc(with_bias=with_bias)
        if not nc.is_finalized():
            nc.finalize()
        _NC[with_bias] = nc
    return _NC[with_bias]


def _host_prep(inputs):
    f32 = np.float32
    bf = ml_dtypes.bfloat16
    g = {k: np.asarray(v) for k, v in inputs.items()}
    # permutation: device ca column c_dev  <->  ref column c_ref = d*16+h
    cdev = np.arange(HID)
    hp_t, p_t = cdev // 128, cdev % 128
    h_t = 2 * hp_t + (p_t // 64)
    d_t = p_t % 64
    pr = d_t * H + h_t                   # ref col for each (ct,partition) row
    consts = {}
    for t, (Wp, bp, g1, be1) in (
            ("c", ("W_cproj", "b_cproj", "g1", "be1")),
            ("m", ("W_mproj", "b_mproj", "g2", "be2"))):
        W = np.asarray(g[Wp], f32)[pr, :]          # [HID, CD] permuted
        g1d = np.asarray(g[g1], f32)[pr]
        be1d = np.asarray(g[be1], f32)[pr]
        consts[f"Wg_{t}"] = np.ascontiguousarray(
            (g1d[:, None] * W)).astype(bf)
        consts[f"v_{t}"] = (be1d @ W + np.asarray(g[bp], f32)).reshape(1, CD)\
            .astype(f32)
        consts[f"un_{t}"] = (-(g1d[:, None] * W).sum(0)).reshape(1, CD)\
            .astype(bf)
    consts["Wq_c"] = np.asarray(g["W_cqkv"], f32).astype(bf)
    consts["Wq_m"] = np.asarray(g["W_mqkv"], f32).astype(bf)
    consts["bq_c"] = np.asarray(g["b_cqkv"], f32).reshape(1, 3 * HID)
    consts["bq_m"] = np.asarray(g["b_mqkv"], f32).reshape(1, 3 * HID)
    p = np.arange(128)
    consts["mask01"] = np.where(
        (p[:, None] // H) == (p[None, :] // H), 1.0, 0.0).astype(bf)
    consts["identb"] = np.eye(128).astype(bf)
    consts["ones_bf"] = np.ones((128, 1)).astype(bf)
    consts["onesr_bf"] = np.ones((1, 128)).astype(bf)
    consts["onesr_f"] = np.ones((1, 128)).astype(f32)
    return g, consts


def kernel(**inputs):
    g, consts = _host_prep(inputs)
    xc = np.ascontiguousarray(np.asarray(g["cnn_out"], np.float32))
    xm = np.ascontiguousarray(np.asarray(g["mlp_out"], np.float32))
    wb = (np.abs(consts["bq_c"]).max() > 0 or np.abs(consts["bq_m"]).max() > 0
          or np.abs(consts["v_c"]).max() > 0 or np.abs(consts["v_m"]).max() > 0)
    nc = _get_nc(bool(wb))
    in_maps = []
    for i in range(NCORES):
        m = dict(consts)
        m["x_c"] = xc[i * BS:(i + 1) * BS]
        m["x_m"] = xm[i * BS:(i + 1) * BS]
        in_maps.append(m)
    res = run_bass_kernel_spmd(nc, in_maps, list(range(NCORES))).results
    out_c = np.concatenate([np.asarray(res[i]["out_c"]) for i in range(NCORES)], 0)
    out_m = np.concatenate([np.asarray(res[i]["out_m"]) for i in range(NCORES)], 0)
    return (out_c.astype(np.float32), out_m.astype(np.float32))

